# revision 4
# baseline (speedup 1.0000x reference)
"""ChamferLoss2D on 8 Trainium2 NeuronCores.

Data parallel: batch n -> core n. Each core computes, for its (4096,2)
point sets x,y, the full 4096x4096 squared-distance matrix via a single
K-row matmul using the norm expansion

    D[i,j] = ||x_i||^2 + ||y_j||^2 - 2 x_i . y_j

then min-reduces it along both axes, sqrts the 2*4096 minima (min and
sqrt commute on nonneg values), means, and averages.

Numerics: the norm expansion cancels catastrophically (D_min ~ 1e-4 vs
terms ~ 1-10), so matmul operands must carry ~fp32 precision. The PE
runs fp32 at 1/4 rate, so instead each fp32 operand row is split
exactly into hi+lo float16 pieces (11-bit mantissa each => products
carry ~22-bit precision) and the matmul runs with K=10 fp16 rows at
full PE speed:

    (xh+xl)(sh+sl) ~ xh.sh + xh.sl + xl.sh        (3 rows per coord)
    norms: (xnh + xnl) * 1, 1 * (ynh + ynl)       (4 rows)

Coordinates are pre-scaled by 16 (exact power of two, folded out of the
final result) so that fp16 subnormal flushing of lo pieces is
negligible relative to D_min.

The distance matrix is consumed from PSUM by the scalar engine
(converting to bf16 in SBUF), and both min reductions run on the vector
engine in bf16 (2x DVE mode): column mins via elementwise tensor-tensor
min accumulation, row mins via in-place halving fold trees.
"""

import os
from contextlib import ExitStack

import numpy as np

import concourse.bass as bass
import concourse.tile as tile
from concourse import bacc, mybir
from concourse.bass_utils import run_bass_kernel_spmd

F32 = mybir.dt.float32
F16 = mybir.dt.float16
BF16 = mybir.dt.bfloat16

P = 4096          # points per set
NCORES = 8
IC = 128          # rows per i-chunk (matmul M)
NI = P // IC      # 32 i-chunks
JW = 1024         # columns per psum tile (2 PSUM banks)
NJ = P // JW      # 4 j-chunks
MMN = 512         # matmul moving free dim (1 bank fp32)

SC = 16.0         # coordinate prescale (power of two)
MIN_OP = mybir.AluOpType.min
ADD_OP = mybir.AluOpType.add
AX = mybir.AxisListType.X


def _build_side(nc, pools, raw, coord_scale, mm_mode):
    """Load one (P,2) point set, build its flat matmul operand rows.

    Returns a dict name -> wide [32,128] tile holding each piece
    (already engine-computed, contiguous), plus the f32 norm row pieces.
    Pieces are later bounced through DRAM into flat [K,P] operand rows.

    coord_scale: SC for the x side, -2*SC for the y side.
    """
    wide, _ = pools
    # [32, 256] f32: partition a holds points a*128..a*128+127, coords
    # interleaved along free axis.
    xw = wide.tile([32, 2 * IC], F32, name=f"xw_{raw.name}")
    nc.sync.dma_start(xw[:], raw.rearrange("(a b) c -> a (b c)", a=32))
    xp = wide.tile([32, 2 * IC], F32, name=f"xp_{raw.name}")
    nc.scalar.mul(xp[:], xw[:], coord_scale)

    # strided per-coordinate views [32, 128]
    xpv = xp[:].rearrange("p (q c) -> p c q", c=2)

    pieces = {}
    # norm row: sum of squares of the *scaled* coords, rescaled so the
    # row holds ||SC*v||^2 regardless of coord_scale.
    sq0 = wide.tile([32, IC], F32, name=f"sq0_{raw.name}")
    nc.vector.tensor_mul(sq0[:], xpv[:, 0, :], xpv[:, 0, :])
    sq1 = wide.tile([32, IC], F32, name=f"sq1_{raw.name}")
    nc.vector.tensor_mul(sq1[:], xpv[:, 1, :], xpv[:, 1, :])
    xn = wide.tile([32, IC], F32, name=f"xn_{raw.name}")
    nc.vector.tensor_add(xn[:], sq0[:], sq1[:])
    norm_fix = (SC / coord_scale) ** 2
    if norm_fix != 1.0:
        nc.scalar.mul(xn[:], xn[:], norm_fix)

    if mm_mode == "f32":
        for c in (0, 1):
            t = wide.tile([32, IC], F32, name=f"c{c}_{raw.name}")
            nc.vector.tensor_copy(t[:], xpv[:, c, :])
            pieces[f"c{c}"] = t
        pieces["n"] = xn
        return pieces

    # fp16 hi/lo split of each coordinate row and of the norm row
    for c in (0, 1):
        h = wide.tile([32, IC], F16, name=f"c{c}h_{raw.name}")
        nc.scalar.copy(h[:], xpv[:, c, :])
        d = wide.tile([32, IC], F32, name=f"c{c}d_{raw.name}")
        nc.vector.tensor_sub(d[:], xpv[:, c, :], h[:])
        l = wide.tile([32, IC], F16, name=f"c{c}l_{raw.name}")
        nc.scalar.copy(l[:], d[:])
        pieces[f"c{c}h"] = h
        pieces[f"c{c}l"] = l
    nh = wide.tile([32, IC], F16, name=f"nh_{raw.name}")
    nc.scalar.copy(nh[:], xn[:])
    nd = wide.tile([32, IC], F32, name=f"nd_{raw.name}")
    nc.vector.tensor_sub(nd[:], xn[:], nh[:])
    nl = wide.tile([32, IC], F16, name=f"nl_{raw.name}")
    nc.scalar.copy(nl[:], nd[:])
    pieces["nh"] = nh
    pieces["nl"] = nl
    return pieces


def _flatten_pieces(nc, dram_pool, flat, placement, pieces):
    """Bounce wide [32,128] pieces through DRAM into rows of `flat`.

    placement: list of (row_index, piece_name). Each piece is written
    once to DRAM and read back once per destination row.
    """
    dt = flat.dtype
    staged = {}
    for _, name in placement:
        if name in staged:
            continue
        d = dram_pool.tile([32, IC], dt, name=f"stage_{flat.name}_{name}")
        nc.sync.dma_start(d[:], pieces[name][:])
        staged[name] = d
    for row, name in placement:
        nc.sync.dma_start(
            flat[row : row + 1, :],
            staged[name][:].rearrange("a b -> () (a b)"),
        )


def build(mm_mode="fp16x2"):
    nc = bacc.Bacc(
        "TRN2", target_bir_lowering=False, debug=False, num_devices=NCORES
    )
    x = nc.dram_tensor("x", [P, 2], F32, kind="ExternalInput").ap()
    y = nc.dram_tensor("y", [P, 2], F32, kind="ExternalInput").ap()
    out_d = nc.dram_tensor("out", [1, 1], F32, kind="ExternalOutput").ap()

    K = 4 if mm_mode == "f32" else 10
    ODT = F32 if mm_mode == "f32" else F16

    with ExitStack() as ctx:
        tc = ctx.enter_context(tile.TileContext(nc))
        konst = ctx.enter_context(tc.tile_pool(name="konst", bufs=1))
        wide = ctx.enter_context(tc.tile_pool(name="wide", bufs=1))
        dram = ctx.enter_context(tc.tile_pool(name="dram", bufs=1, space="DRAM"))
        psum = ctx.enter_context(tc.tile_pool(name="psum", bufs=3, space="PSUM"))
        fpsum = ctx.enter_context(tc.tile_pool(name="fpsum", bufs=1, space="PSUM"))
        rows = ctx.enter_context(tc.tile_pool(name="rows", bufs=3))
        accp = ctx.enter_context(tc.tile_pool(name="accp", bufs=1))
        smalls = ctx.enter_context(tc.tile_pool(name="smalls", bufs=1))

        lhsT = konst.tile([K, P], ODT, name="lhsT")
        rhs = konst.tile([K, P], ODT, name="rhs")
        colacc = accp.tile([IC, P], BF16, name="colacc")
        nc.gpsimd.memset(colacc[:], 1.0e30)
        rmins = smalls.tile([IC, NI], F32, name="rmins")

        xp = _build_side(nc, (wide, None), x, SC, mm_mode)
        yp = _build_side(nc, (wide, None), y, -2.0 * SC, mm_mode)

        # memset whole operand tiles to 1.0 (the "ones" rows), then DMA
        # the data rows over them (gpsimd memset must start at part. 0)
        nc.gpsimd.memset(lhsT[:], 1.0)
        nc.gpsimd.memset(rhs[:], 1.0)
        if mm_mode == "f32":
            # lhsT rows: [x0, x1, xn, 1]; rhs rows: [s0, s1, 1, yn]
            _flatten_pieces(nc, dram, lhsT,
                            [(0, "c0"), (1, "c1"), (2, "n")], xp)
            _flatten_pieces(nc, dram, rhs,
                            [(0, "c0"), (1, "c1"), (3, "n")], yp)
        else:
            # lhsT rows: [x0h x0h x0l  x1h x1h x1l  xnh xnl  1 1]
            # rhs  rows: [s0h s0l s0h  s1h s1l s1h  1   1    ynh ynl]
            _flatten_pieces(nc, dram, lhsT,
                            [(0, "c0h"), (1, "c0h"), (2, "c0l"),
                             (3, "c1h"), (4, "c1h"), (5, "c1l"),
                             (6, "nh"), (7, "nl")], xp)
            _flatten_pieces(nc, dram, rhs,
                            [(0, "c0h"), (1, "c0l"), (2, "c0h"),
                             (3, "c1h"), (4, "c1l"), (5, "c1h"),
                             (8, "nh"), (9, "nl")], yp)

        # ---- main loop: D tiles -> bf16 -> col/row min reductions ----
        for ic in range(NI):
            lt = lhsT[:, ic * IC : (ic + 1) * IC]
            drow = rows.tile([IC, P], BF16, name="drow")
            for jc in range(NJ):
                pt = psum.tile([IC, JW], F32, name="pt")
                for h in range(JW // MMN):
                    j0 = jc * JW + h * MMN
                    nc.tensor.matmul(
                        pt[:, h * MMN : (h + 1) * MMN],
                        lt,
                        rhs[:, j0 : j0 + MMN],
                        start=True,
                        stop=True,
                    )
                js = slice(jc * JW, (jc + 1) * JW)
                # f32 PSUM -> bf16 SBUF (scalar engine)
                nc.scalar.copy(drow[:, js], pt[:])
                # column-min accumulation (elementwise, bf16 2x mode)
                nc.vector.tensor_tensor(
                    colacc[:, js], colacc[:, js], drow[:, js], op=MIN_OP
                )
            # row mins: in-place halving fold tree down to 32 wide
            w = P
            while w > 32:
                w //= 2
                nc.vector.tensor_tensor(
                    drow[:, :w], drow[:, :w], drow[:, w : 2 * w], op=MIN_OP
                )
            nc.vector.tensor_reduce(
                rmins[:, ic : ic + 1], drow[:, :32], axis=AX, op=MIN_OP
            )

        # ---- finalize row-direction: clamp, sqrt, sum over free ----
        rclamp = smalls.tile([IC, NI], F32, name="rclamp")
        nc.vector.tensor_scalar_max(rclamp[:], rmins[:], 0.0)
        rsq = smalls.tile([IC, NI], F32, name="rsq")
        nc.scalar.activation(rsq[:], rclamp[:], mybir.ActivationFunctionType.Sqrt)
        rsum = smalls.tile([IC, 1], F32, name="rsum")
        nc.vector.tensor_reduce(rsum[:], rsq[:], axis=AX, op=ADD_OP)

        # ---- finalize col-direction ----
        # DVE cannot reduce across partitions; DMA-xbar-transpose the
        # accumulated [128, P] bf16 colacc in 128x128 blocks, then one
        # 3D reduce-min over each block's free axis gives colmin as
        # [128, NI] (partition p, block b -> column b*128+p).
        ct = accp.tile([IC, P], BF16, name="ct")
        for b in range(P // IC):
            bs = slice(b * IC, (b + 1) * IC)
            nc.sync.dma_start(ct[:, bs], colacc[:, bs], transpose=True)
        cmin = smalls.tile([IC, P // IC], F32, name="cmin")
        nc.vector.tensor_reduce(
            cmin[:], ct[:].rearrange("p (b q) -> p b q", q=IC), axis=AX, op=MIN_OP
        )
        cclamp = smalls.tile([IC, P // IC], F32, name="cclamp")
        nc.vector.tensor_scalar_max(cclamp[:], cmin[:], 0.0)
        csq = smalls.tile([IC, P // IC], F32, name="csq")
        nc.scalar.activation(csq[:], cclamp[:], mybir.ActivationFunctionType.Sqrt)
        csum = smalls.tile([IC, 1], F32, name="csum")
        nc.vector.tensor_reduce(csum[:], csq[:], axis=AX, op=ADD_OP)

        # ---- combine: partition-sum via matmul with ones, then scale ----
        both = smalls.tile([IC, 1], F32, name="both")
        nc.vector.tensor_add(both[:], rsum[:], csum[:])
        ones = smalls.tile([IC, 1], F32, name="ones")
        nc.gpsimd.memset(ones[:], 1.0)
        tot_ps = fpsum.tile([1, 1], F32, name="tot_ps")
        nc.tensor.matmul(tot_ps[:], both[:], ones[:], start=True, stop=True)
        fin = smalls.tile([1, 1], F32, name="fin")
        nc.scalar.mul(fin[:], tot_ps[:], 1.0 / (SC * 2.0 * P))
        nc.sync.dma_start(out_d, fin[:])

    nc.compile()
    return nc


_NC_CACHE = {}


def _get_nc(mm_mode):
    if mm_mode not in _NC_CACHE:
        _NC_CACHE[mm_mode] = build(mm_mode)
    return _NC_CACHE[mm_mode]


def run(point_set_1, point_set_2, mm_mode=None, trace=False, tmpdir=None):
    """Run on 8 cores; returns ((8,) result, BassKernelResults)."""
    mm_mode = mm_mode or os.environ.get("CHAMFER_MM_MODE", "fp16x2")
    nc = _get_nc(mm_mode)
    x = np.ascontiguousarray(np.asarray(point_set_1), dtype=np.float32)
    y = np.ascontiguousarray(np.asarray(point_set_2), dtype=np.float32)
    assert x.shape == (NCORES, P, 2) and y.shape == (NCORES, P, 2)
    in_maps = [{"x": x[c], "y": y[c]} for c in range(NCORES)]
    res = run_bass_kernel_spmd(
        nc, in_maps, list(range(NCORES)), trace=trace, tmpdir=tmpdir
    )
    out = np.array(
        [res.results[c]["out"][0, 0] for c in range(NCORES)], dtype=np.float32
    )
    return out, res


def kernel(point_set_1, point_set_2):
    out, _ = run(point_set_1, point_set_2)
    return out
